# revision 16
# baseline (speedup 1.0000x reference)
"""Trainium2 Bass kernel for nn_LocalGroupedZernikeNewBP.

Full inputs in, full output out. Shards raw_coeffs [8,256,256,36] along the
batch dim: one image per NeuronCore (8 cores). Scalar params are baked into
the compiled program as immediates (rebuilt per distinct param values).

Per-core program (image [H=256, W=256, C=36], HWC contiguous):
  - 2 row-blocks of 128 rows x 2 column-halves of 128 cols (4 work units,
    SBUF tiles [128, 128*36]; DMAs contiguous 18KB rows).
  - special (ch 0:3):  out = amax * tanh(alpha*(x+bias))
  - joint groups low(3:6) mid(6:15) high(15:36):
      soft_abs = sqrt((x+bias)^2 + eps)     (GPSIMD square when bias==0,
                                             else ACT Square; ACT Sqrt)
      chansum  = sum_c soft_abs             (DVE reduce over C)
      s        = 3x3 box(chansum), edge-replicated (DVE shifted adds; row
                 shifts via SBUF->SBUF DMA, halo between the row-blocks)
      G        = alpha*gss/(1 + s/p_sat)   (DVE, one reciprocal per block)
      v        = (x+bias) * G               (DVE scalar_tensor_tensor,
                                             G broadcast over C)
      out      = amax * tanh(v)             (ACT Tanh whole tile; amax via
                                             DVE ch 0:15 + ACT ch 15:36)
"""

import numpy as np

B, H, W, C = 8, 256, 256, 36
NWU = 4              # column work-units per row-block
WH = W // NWU        # column-unit width
FD = W * C           # free elements per full row
FDH = WH * C         # free elements per column unit
GROUPS = [("low", 3, 6), ("mid", 6, 15), ("high", 15, 36)]
N_CORES = 8

_NC_CACHE: dict[tuple, object] = {}


def _build(p: dict[str, float]):
    from contextlib import ExitStack

    import concourse.bass as bass  # noqa: F401
    import concourse.tile as tile
    from concourse import bacc, mybir

    f32 = mybir.dt.float32
    AF = mybir.ActivationFunctionType
    OP = mybir.AluOpType
    AX = mybir.AxisListType

    nc = bacc.Bacc(
        "TRN2", target_bir_lowering=False, debug=False, num_devices=N_CORES
    )
    x = nc.dram_tensor("x", [H, FD], f32, kind="ExternalInput").ap()
    bands = nc.dram_tensor("bands", [128, 256], f32,
                           kind="ExternalInput").ap()
    halos = nc.dram_tensor("halos", [1, 256], f32, kind="ExternalInput").ap()
    y = nc.dram_tensor("y", [H, FD], f32, kind="ExternalOutput").ap()

    joint_bias_zero = all(p[g + "_bias"] == 0.0 for g, _, _ in GROUPS)

    with tile.TileContext(nc) as tc, ExitStack() as ctx:
        xp = ctx.enter_context(tc.tile_pool(name="xp", bufs=2 * NWU))
        sqp = ctx.enter_context(tc.tile_pool(name="sqp", bufs=3))
        mp = ctx.enter_context(tc.tile_pool(name="maps", bufs=1))
        cp = ctx.enter_context(tc.tile_pool(name="consts", bufs=1))
        psp = ctx.enter_context(tc.tile_pool(name="psum", bufs=3,
                                             space="PSUM"))

        _consts: dict[float, object] = {}

        def constant(val: float):
            """[128,1] SBUF tile holding `val` (for ACT bias operands)."""
            val = float(val)
            if val not in _consts:
                ct = cp.tile([128, 1], f32, tag=f"const{len(_consts)}")
                nc.vector.memset(ct[:], val)
                _consts[val] = ct
            return _consts[val][:]

        xt = {}
        cs = {}
        Tm = {}
        Sall = {}

        # Phase A: load, soft_abs, channel sums (per row-block x col-half)
        for rb in range(2):
            for g, _, _ in GROUPS:
                cs[(rb, g)] = mp.tile([128, W], f32, tag=f"cs{rb}{g}",
                                      name=f"cs{rb}{g}")
            for h in range(NWU):
                t = xp.tile([128, FDH], f32, tag="x")
                nc.sync.dma_start(
                    t[:], x[rb * 128 : (rb + 1) * 128,
                            h * FDH : (h + 1) * FDH])
                xt[(rb, h)] = t
                x3 = t[:].rearrange("p (w c) -> p w c", c=C)
                sq = sqp.tile([128, FDH], f32, tag="sq")
                sq3 = sq[:].rearrange("p (w c) -> p w c", c=C)
                if joint_bias_zero:
                    # one contiguous Square over ch 3:36 (GPSIMD would be
                    # faster in isolation but steals DVE SBUF ports)
                    nc.scalar.activation(
                        sq[:].rearrange("p (w c) -> p w c", c=C)[:, :, 3:36],
                        x3[:, :, 3:36], AF.Square)
                else:
                    for g, c0, c1 in GROUPS:
                        nc.scalar.activation(sq3[:, :, c0:c1],
                                             x3[:, :, c0:c1], AF.Square,
                                             bias=constant(p[g + "_bias"]))
                if len({p[g + "_eps"] for g, _, _ in GROUPS}) == 1:
                    nc.scalar.activation(sq3[:, :, 3:36], sq3[:, :, 3:36],
                                         AF.Sqrt,
                                         bias=constant(p["low_eps"]))
                else:
                    for g, c0, c1 in GROUPS:
                        nc.scalar.activation(sq3[:, :, c0:c1],
                                             sq3[:, :, c0:c1], AF.Sqrt,
                                             bias=constant(p[g + "_eps"]))
                for g, c0, c1 in GROUPS:
                    nc.vector.reduce_sum(
                        cs[(rb, g)][:, h * WH : (h + 1) * WH],
                        sq3[:, :, c0:c1], axis=AX.X)

            # W-direction 3-tap box with replicate edges (full width)
            for g, c0, c1 in GROUPS:
                c_ = cs[(rb, g)]
                T = mp.tile([128, W], f32, tag=f"T{rb}{g}")
                nc.vector.tensor_add(T[:, 1 : W - 1], c_[:, 0 : W - 2],
                                     c_[:, 2:W])
                nc.vector.tensor_add(T[:, 0:1], c_[:, 0:1], c_[:, 1:2])
                nc.vector.tensor_add(T[:, W - 1 : W], c_[:, W - 2 : W - 1],
                                     c_[:, W - 1 : W])
                nc.vector.tensor_add(T[:], T[:], c_[:])
                Tm[(rb, g)] = T

        # Phase A2: H-direction 3-tap box on TensorE: S = band.T @ T plus a
        # rank-1 halo matmul for the row from the other block (replicate
        # edges are baked into the band matrices, passed in as inputs).
        # PE rhs partition APs must start at 0/32/64/96, so block 1's halo
        # row (T0 row 127) is first DMA-copied to partition 0 of a scratch
        # tile. The 3 group maps of a block share one SBUF tile so one
        # reciprocal per block gives
        # G = 1/(s/(p_sat*alpha*gss) + 1/(alpha*gss)) = alpha*gss/(1+s/p_sat).
        bands_t = cp.tile([128, 256], f32, tag="bands")
        halos_t = cp.tile([1, 256], f32, tag="halos")
        nc.sync.dma_start(bands_t[:], bands[:])
        nc.sync.dma_start(halos_t[:], halos[:])
        hrow = {}
        for g, c0, c1 in GROUPS:
            hr = mp.tile([1, W], f32, tag=f"hrow{g}", name=f"hrow{g}")
            nc.gpsimd.dma_start(hr[:], Tm[(0, g)][127:128, :])
            hrow[g] = hr
        rscratch = mp.tile([128, 3 * W], f32, tag="rscratch")
        for rb in range(2):
            Sa = mp.tile([128, 3 * W], f32, tag=f"Sall{rb}")
            Sall[rb] = Sa
            for gi, (g, c0, c1) in enumerate(GROUPS):
                S_ps = psp.tile([128, W], f32, tag="ps")
                nc.tensor.matmul(S_ps[:],
                                 bands_t[:, rb * 128 : (rb + 1) * 128],
                                 Tm[(rb, g)][:], start=True, stop=False)
                halo_rhs = Tm[(1, g)][0:1, :] if rb == 0 else hrow[g][:]
                nc.tensor.matmul(S_ps[:],
                                 halos_t[0:1, rb * 128 : (rb + 1) * 128],
                                 halo_rhs, start=False, stop=True)
                S = Sa[:, gi * W : (gi + 1) * W]
                ags = p[g + "_alpha"] * p[g + "_gss"]
                nc.vector.tensor_scalar(S, S_ps[:],
                                        float(1.0 / (p[g + "_p_sat"] * ags)),
                                        float(1.0 / ags),
                                        op0=OP.mult, op1=OP.add)
            nc.vector.reciprocal_approx_accurate(Sa[:], Sa[:],
                                                 rscratch[:])

        # Phase B: v = (x+bias)/r in place, tanh, amax, store
        for rb in range(2):
            for h in range(NWU):
                t = xt[(rb, h)]
                x3 = t[:].rearrange("p (w c) -> p w c", c=C)
                sp = x3[:, :, 0:3]
                if p["special_alpha"] != 1.0 or p["special_bias"] != 0.0:
                    nc.scalar.activation(
                        sp, sp, AF.Copy,
                        bias=float(p["special_alpha"] * p["special_bias"]),
                        scale=float(p["special_alpha"]))
                for gi, (g, c0, c1) in enumerate(GROUPS):
                    cg = c1 - c0
                    rsl = Sall[rb][:, gi * W + h * WH : gi * W + (h + 1) * WH]
                    rb_ = rsl.unsqueeze(2).to_broadcast([128, WH, cg])
                    nc.vector.scalar_tensor_tensor(
                        x3[:, :, c0:c1], x3[:, :, c0:c1],
                        float(p[g + "_bias"]), rb_,
                        op0=OP.add, op1=OP.mult)
                nc.scalar.activation(t[:], t[:], AF.Tanh)
                # final amax scale (skipped when amax == 1):
                # DVE ch 0:6, ACT ch 6:36
                if p["special_amax"] != 1.0:
                    nc.vector.tensor_scalar_mul(x3[:, :, 0:3], x3[:, :, 0:3],
                                                float(p["special_amax"]))
                if p["low_amax"] != 1.0:
                    nc.vector.tensor_scalar_mul(x3[:, :, 3:6], x3[:, :, 3:6],
                                                float(p["low_amax"]))
                if p["mid_amax"] != 1.0:
                    nc.scalar.mul(x3[:, :, 6:15], x3[:, :, 6:15],
                                  float(p["mid_amax"]))
                if p["high_amax"] != 1.0:
                    nc.scalar.mul(x3[:, :, 15:36], x3[:, :, 15:36],
                                  float(p["high_amax"]))
                nc.sync.dma_start(
                    y[rb * 128 : (rb + 1) * 128, h * FDH : (h + 1) * FDH],
                    t[:])

    nc.compile()
    return nc


_SCALARS = [
    "special_bias", "special_alpha", "special_amax", "special_eps",
    "low_bias", "low_alpha", "low_amax", "low_eps", "low_gss", "low_p_sat",
    "mid_bias", "mid_alpha", "mid_amax", "mid_eps", "mid_gss", "mid_p_sat",
    "high_bias", "high_alpha", "high_amax", "high_eps", "high_gss",
    "high_p_sat",
]


def build_nc(**inputs):
    """Build (or fetch cached) compiled Bass program for these scalar params."""
    p = {k: float(np.asarray(inputs[k]).reshape(-1)[0]) for k in _SCALARS}
    key = tuple(p[k] for k in _SCALARS)
    if key not in _NC_CACHE:
        _NC_CACHE[key] = _build(p)
    return _NC_CACHE[key]


def _band_arrays():
    """Band matrices (lhsT, [k, m] = contribution of input row k to output
    row m) for the H-direction 3-tap box, replicate edges baked in, plus
    rank-1 halo row selectors."""
    A = np.zeros((128, 128), np.float32)
    for m in range(128):
        for k in (m - 1, m, m + 1):
            if 0 <= k < 128:
                A[k, m] = 1.0
    A0 = A.copy()
    A0[0, 0] = 2.0       # top replicate (block 0)
    A1 = A.copy()
    A1[127, 127] = 2.0   # bottom replicate (block 1)
    bands = np.concatenate([A0, A1], axis=1)          # [128, 256]
    halos = np.zeros((1, 256), np.float32)
    halos[0, 127] = 1.0      # block 0: out[127] += T1[0]
    halos[0, 128 + 0] = 1.0  # block 1: out[0]   += T0[127]
    return bands, halos


def kernel(**inputs) -> np.ndarray:
    from concourse.bass_utils import run_bass_kernel_spmd

    raw = np.ascontiguousarray(np.asarray(inputs["raw_coeffs"],
                                          dtype=np.float32))
    assert raw.shape == (B, H, W, C), raw.shape
    nc = build_nc(**inputs)
    bands, halos = _band_arrays()
    in_maps = [{"x": raw[i].reshape(H, FD), "bands": bands, "halos": halos}
               for i in range(N_CORES)]
    res = run_bass_kernel_spmd(nc, in_maps, list(range(N_CORES)))
    out = np.stack([res.results[i]["y"].reshape(H, W, C)
                    for i in range(N_CORES)])
    return out.astype(np.float32)


# revision 19
# speedup vs baseline: 1.0427x; 1.0427x over previous
"""Trainium2 Bass kernel for nn_LocalGroupedZernikeNewBP.

Full inputs in, full output out. Shards raw_coeffs [8,256,256,36] along the
batch dim: one image per NeuronCore (8 cores). Scalar params are baked into
the compiled program as immediates (rebuilt per distinct param values).

Per-core program (image [H=256, W=256, C=36], HWC contiguous):
  - 2 row-blocks of 128 rows x NWU column units (SBUF tiles [128, WH*36],
    contiguous-row DMAs).
  - special (ch 0:3):  out = amax * tanh(alpha*(x+bias))
  - joint groups low(3:6) mid(6:15) high(15:36):
      soft_abs = sqrt((x+bias)^2 + eps)     (ACT Square + ACT Sqrt)
      chansum  = sum_c soft_abs             (DVE reduce over C)
      s        = 3x3 box(chansum), replicate edges:
                   W direction: DVE shifted adds along the free dim
                   H direction: TensorE band matmul into PSUM + rank-1 halo
                   matmul. The halo row of the OTHER block is computed by a
                   small dedicated path (boundary image rows 127/128 loaded
                   as [128, 72] tiles, reduced, DMA-gathered to a [1,256]
                   line) so each block's phase B only depends on its own
                   phase A - the two blocks pipeline instead of joining.
      G        = alpha*gss/(1 + s/p_sat)    (DVE tensor_scalar +
                                             reciprocal_approx_accurate)
      v        = (x+bias) * G               (DVE scalar_tensor_tensor,
                                             G broadcast over C)
      out      = amax * tanh(v)             (ACT Tanh whole tile; identity
                                             amax/prescale ops are skipped)
"""

import numpy as np

B, H, W, C = 8, 256, 256, 36
NWU = 4              # column work-units per row-block
WH = W // NWU        # column-unit width
FD = W * C           # free elements per full row
FDH = WH * C         # free elements per column unit
GROUPS = [("low", 3, 6), ("mid", 6, 15), ("high", 15, 36)]
N_CORES = 8

_NC_CACHE: dict[tuple, object] = {}


def _build(p: dict[str, float]):
    from contextlib import ExitStack

    import concourse.bass as bass  # noqa: F401
    import concourse.tile as tile
    from concourse import bacc, mybir

    f32 = mybir.dt.float32
    AF = mybir.ActivationFunctionType
    OP = mybir.AluOpType
    AX = mybir.AxisListType

    nc = bacc.Bacc(
        "TRN2", target_bir_lowering=False, debug=False, num_devices=N_CORES
    )
    x = nc.dram_tensor("x", [H, FD], f32, kind="ExternalInput").ap()
    bands = nc.dram_tensor("bands", [128, 256], f32,
                           kind="ExternalInput").ap()
    halos = nc.dram_tensor("halos", [1, 256], f32, kind="ExternalInput").ap()
    y = nc.dram_tensor("y", [H, FD], f32, kind="ExternalOutput").ap()

    joint_bias_zero = all(p[g + "_bias"] == 0.0 for g, _, _ in GROUPS)
    eps_equal = len({p[g + "_eps"] for g, _, _ in GROUPS}) == 1

    with tile.TileContext(nc) as tc, ExitStack() as ctx:
        xp = ctx.enter_context(tc.tile_pool(name="xp", bufs=2 * NWU))
        sqp = ctx.enter_context(tc.tile_pool(name="sqp", bufs=3))
        mp = ctx.enter_context(tc.tile_pool(name="maps", bufs=1))
        cp = ctx.enter_context(tc.tile_pool(name="consts", bufs=1))
        psp = ctx.enter_context(tc.tile_pool(name="psum", bufs=3,
                                             space="PSUM"))

        _consts: dict[float, object] = {}

        def constant(val: float):
            """[128,1] SBUF tile holding `val` (for ACT bias operands)."""
            val = float(val)
            if val not in _consts:
                ct = cp.tile([128, 1], f32, tag=f"const{len(_consts)}")
                nc.vector.memset(ct[:], val)
                _consts[val] = ct
            return _consts[val][:]

        def soft_abs(dst3, src3):
            """dst3 = sqrt((src3+bias)^2+eps) on ch 3:36; [p, w, 36] APs."""
            if joint_bias_zero:
                nc.scalar.activation(dst3[:, :, 3:36], src3[:, :, 3:36],
                                     AF.Square)
            else:
                for g, c0, c1 in GROUPS:
                    nc.scalar.activation(dst3[:, :, c0:c1], src3[:, :, c0:c1],
                                         AF.Square,
                                         bias=constant(p[g + "_bias"]))
            if eps_equal:
                nc.scalar.activation(dst3[:, :, 3:36], dst3[:, :, 3:36],
                                     AF.Sqrt, bias=constant(p["low_eps"]))
            else:
                for g, c0, c1 in GROUPS:
                    nc.scalar.activation(dst3[:, :, c0:c1], dst3[:, :, c0:c1],
                                         AF.Sqrt,
                                         bias=constant(p[g + "_eps"]))

        def wbox(dst, src, n):
            """3-tap box along the free dim (length n), replicate edges.
            dst/src are [partitions, n] APs on distinct tiles."""
            nc.vector.tensor_add(dst[:, 1 : n - 1], src[:, 0 : n - 2],
                                 src[:, 2:n])
            nc.vector.tensor_add(dst[:, 0:1], src[:, 0:1], src[:, 1:2])
            nc.vector.tensor_add(dst[:, n - 1 : n], src[:, n - 2 : n - 1],
                                 src[:, n - 1 : n])
            nc.vector.tensor_add(dst[:], dst[:], src[:])

        bands_t = cp.tile([128, 256], f32, tag="bands")
        halos_t = cp.tile([1, 256], f32, tag="halos")
        nc.sync.dma_start(bands_t[:], bands[:])
        nc.sync.dma_start(halos_t[:], halos[:])

        # ---- dedicated halo-row path: image rows 127 (for block 1) and
        # 128 (for block 0). Row -> [128, 72] tile (w pairs x 36 ch),
        # soft_abs, per-group reduce to [128, 2], DMA-gather to a [1, 768]
        # line (group-major), W-box -> halo T line [1, 768].
        halo_T = {}
        for hrow, need_rb in ((128, 0), (127, 1)):
            hx = mp.tile([128, 72], f32, tag=f"hx{hrow}", name=f"hx{hrow}")
            nc.sync.dma_start(hx[:], x[hrow : hrow + 1, :])
            hx3 = hx[:].rearrange("p (w c) -> p w c", c=C)
            hsq = mp.tile([128, 72], f32, tag=f"hsq{hrow}", name=f"hsq{hrow}")
            soft_abs(hsq[:].rearrange("p (w c) -> p w c", c=C), hx3)
            hcs = mp.tile([128, 6], f32, tag=f"hcs{hrow}", name=f"hcs{hrow}")
            hsq3 = hsq[:].rearrange("p (w c) -> p w c", c=C)
            for gi, (g, c0, c1) in enumerate(GROUPS):
                nc.vector.reduce_sum(hcs[:, gi * 2 : gi * 2 + 2],
                                     hsq3[:, :, c0:c1], axis=AX.X)
            hline = mp.tile([1, 768], f32, tag=f"hline{hrow}",
                            name=f"hline{hrow}")
            # gather partitions, per group: element (p, j) -> 2*p + j
            for gi in range(3):
                out_ap = hline[0:1, gi * 256 : (gi + 1) * 256].rearrange(
                    "q (p j) -> q p j", j=2)
                nc.gpsimd.dma_start(out_ap, hcs[:, gi * 2 : gi * 2 + 2])
            hT = mp.tile([1, 768], f32, tag=f"hT{hrow}", name=f"hT{hrow}")
            for gi in range(3):
                wbox(hT[0:1, gi * 256 : (gi + 1) * 256],
                     hline[0:1, gi * 256 : (gi + 1) * 256], 256)
            halo_T[need_rb] = hT

        # ---- main pipeline, per row-block ----
        rscratch = mp.tile([128, 3 * W], f32, tag="rscratch")
        for rb in range(2):
            cs = {}
            for g, _, _ in GROUPS:
                cs[g] = mp.tile([128, W], f32, tag=f"cs{rb}{g}",
                                name=f"cs{rb}{g}")
            xt = {}
            # phase A: load + soft_abs + channel sums per column unit
            for h in range(NWU):
                t = xp.tile([128, FDH], f32, tag="x")
                nc.sync.dma_start(
                    t[:], x[rb * 128 : (rb + 1) * 128,
                            h * FDH : (h + 1) * FDH])
                xt[h] = t
                x3 = t[:].rearrange("p (w c) -> p w c", c=C)
                sq = sqp.tile([128, FDH], f32, tag="sq")
                sq3 = sq[:].rearrange("p (w c) -> p w c", c=C)
                soft_abs(sq3, x3)
                for g, c0, c1 in GROUPS:
                    nc.vector.reduce_sum(cs[g][:, h * WH : (h + 1) * WH],
                                         sq3[:, :, c0:c1], axis=AX.X)

            # W-direction box, then H-direction via TensorE band matmul
            # (+ rank-1 halo from the dedicated path), then gain map.
            Sa = mp.tile([128, 3 * W], f32, tag=f"Sall{rb}",
                         name=f"Sall{rb}")
            for gi, (g, c0, c1) in enumerate(GROUPS):
                T = mp.tile([128, W], f32, tag=f"T{rb}{g}", name=f"T{rb}{g}")
                wbox(T[:], cs[g][:], W)
                S_ps = psp.tile([128, W], f32, tag="ps")
                nc.tensor.matmul(S_ps[:],
                                 bands_t[:, rb * 128 : (rb + 1) * 128],
                                 T[:], start=True, stop=False)
                nc.tensor.matmul(
                    S_ps[:], halos_t[0:1, rb * 128 : (rb + 1) * 128],
                    halo_T[rb][0:1, gi * 256 : (gi + 1) * 256],
                    start=False, stop=True)
                S = Sa[:, gi * W : (gi + 1) * W]
                ags = p[g + "_alpha"] * p[g + "_gss"]
                nc.vector.tensor_scalar(S, S_ps[:],
                                        float(1.0 / (p[g + "_p_sat"] * ags)),
                                        float(1.0 / ags),
                                        op0=OP.mult, op1=OP.add)
            nc.vector.reciprocal_approx_accurate(Sa[:], Sa[:], rscratch[:])

            # phase B: v = (x+bias)*G in place, tanh, (amax), store
            for h in range(NWU):
                t = xt[h]
                x3 = t[:].rearrange("p (w c) -> p w c", c=C)
                sp = x3[:, :, 0:3]
                if p["special_alpha"] != 1.0 or p["special_bias"] != 0.0:
                    nc.scalar.activation(
                        sp, sp, AF.Copy,
                        bias=float(p["special_alpha"] * p["special_bias"]),
                        scale=float(p["special_alpha"]))
                for gi, (g, c0, c1) in enumerate(GROUPS):
                    cg = c1 - c0
                    rsl = Sa[:, gi * W + h * WH : gi * W + (h + 1) * WH]
                    gb = rsl.unsqueeze(2).to_broadcast([128, WH, cg])
                    nc.vector.scalar_tensor_tensor(
                        x3[:, :, c0:c1], x3[:, :, c0:c1],
                        float(p[g + "_bias"]), gb, op0=OP.add, op1=OP.mult)
                nc.scalar.activation(t[:], t[:], AF.Tanh)
                # final amax scale (skipped when amax == 1):
                # DVE ch 0:6, ACT ch 6:36
                if p["special_amax"] != 1.0:
                    nc.vector.tensor_scalar_mul(x3[:, :, 0:3], x3[:, :, 0:3],
                                                float(p["special_amax"]))
                if p["low_amax"] != 1.0:
                    nc.vector.tensor_scalar_mul(x3[:, :, 3:6], x3[:, :, 3:6],
                                                float(p["low_amax"]))
                if p["mid_amax"] != 1.0:
                    nc.scalar.mul(x3[:, :, 6:15], x3[:, :, 6:15],
                                  float(p["mid_amax"]))
                if p["high_amax"] != 1.0:
                    nc.scalar.mul(x3[:, :, 15:36], x3[:, :, 15:36],
                                  float(p["high_amax"]))
                nc.sync.dma_start(
                    y[rb * 128 : (rb + 1) * 128, h * FDH : (h + 1) * FDH],
                    t[:])

    nc.compile()
    return nc


_SCALARS = [
    "special_bias", "special_alpha", "special_amax", "special_eps",
    "low_bias", "low_alpha", "low_amax", "low_eps", "low_gss", "low_p_sat",
    "mid_bias", "mid_alpha", "mid_amax", "mid_eps", "mid_gss", "mid_p_sat",
    "high_bias", "high_alpha", "high_amax", "high_eps", "high_gss",
    "high_p_sat",
]


def build_nc(**inputs):
    """Build (or fetch cached) compiled Bass program for these scalar params."""
    p = {k: float(np.asarray(inputs[k]).reshape(-1)[0]) for k in _SCALARS}
    key = tuple(p[k] for k in _SCALARS)
    if key not in _NC_CACHE:
        _NC_CACHE[key] = _build(p)
    return _NC_CACHE[key]


def _band_arrays():
    """Band matrices (lhsT, [k, m] = contribution of input row k to output
    row m) for the H-direction 3-tap box, replicate edges baked in, plus
    rank-1 halo row selectors."""
    A = np.zeros((128, 128), np.float32)
    for m in range(128):
        for k in (m - 1, m, m + 1):
            if 0 <= k < 128:
                A[k, m] = 1.0
    A0 = A.copy()
    A0[0, 0] = 2.0       # top replicate (block 0)
    A1 = A.copy()
    A1[127, 127] = 2.0   # bottom replicate (block 1)
    bands = np.concatenate([A0, A1], axis=1)          # [128, 256]
    halos = np.zeros((1, 256), np.float32)
    halos[0, 127] = 1.0      # block 0: out[127] += T(row 128)
    halos[0, 128 + 0] = 1.0  # block 1: out[0]   += T(row 127)
    return bands, halos


def kernel(**inputs) -> np.ndarray:
    from concourse.bass_utils import run_bass_kernel_spmd

    raw = np.ascontiguousarray(np.asarray(inputs["raw_coeffs"],
                                          dtype=np.float32))
    assert raw.shape == (B, H, W, C), raw.shape
    nc = build_nc(**inputs)
    bands, halos = _band_arrays()
    in_maps = [{"x": raw[i].reshape(H, FD), "bands": bands, "halos": halos}
               for i in range(N_CORES)]
    res = run_bass_kernel_spmd(nc, in_maps, list(range(N_CORES)))
    out = np.stack([res.results[i]["y"].reshape(H, W, C)
                    for i in range(N_CORES)])
    return out.astype(np.float32)
